# revision 60
# baseline (speedup 1.0000x reference)
"""EntropyGuidance Trainium2 kernel — bf16 I/O, transpose-then-exp, staggered.

Layout: per sample [128 partitions = (h c): p = h*64 + c, 8192 free] bf16.
DRAM output is bf16 (host upcasts), loads cast f32->bf16 in the DGE, so
every DMA is billed on its 2-byte side. fp32 is kept for statistics.

Per 512-column group g of each sample:
  1. PE transposes raw vis|text subchunks into one PSUM tile [128, 1024].
  2. exp reads the PSUM tile and writes transposed exponentials straight
     to SBUF: on the Activation engine normally, or as a one-op DVE
     Schraudolph (i16 = 184.6627*t + 16250.4, bitcast to bf16; ~3% rel
     err, only ever used by the guide-weight path which tolerates it)
     for groups listed in DVE_EXP — this drains the Act queue faster.
  3. J += evT^T @ etT per 128-column slice (PSUM fp32).
  4. Sv/St/T ride near-free PE matmuls with a [128,1] ones rhs
     (T uses xeT = rawT_text * etT, one DVE mult per group).

Folds (h c) -> c use a base-partition-changing copy plus a PSUM+SBUF add
(both verified legal); the guide is computed on 64 partitions and
broadcast with two copies. Samples are staggered: s0's phase-2 and
output pass are emitted interleaved into s1's group stream so no engine
queue head-of-line blocks the other sample.
"""

import sys

sys.path.insert(0, "/opt/trn_rl_repo")

from contextlib import ExitStack

import numpy as np

import concourse.bacc as bacc
import concourse.tile as tile
from concourse import mybir
from concourse.bass_utils import run_bass_kernel_spmd
from concourse.masks import make_identity

_orig_get_act_tables = bacc.get_activation_tables


def _lnexp_only_tables(module_arch):
    tabs = _orig_get_act_tables(module_arch)
    return {
        name: (funcs if name == "natural_log_exp_and_others" else set())
        for name, funcs in tabs.items()
    }


bacc.get_activation_tables = _lnexp_only_tables

F32 = mybir.dt.float32
BF16 = mybir.dt.bfloat16
I16 = mybir.dt.int16
AF = mybir.ActivationFunctionType
ALU = mybir.AluOpType
AX = mybir.AxisListType

B, C, H, W = 16, 64, 128, 128
HW = H * W                      # 16384
HH = HW // 2                    # 8192 per half
NCORES = 8
P = 128                         # partitions = (h c)
EPS = 1e-9
GW = 512                        # n-columns per group
NG = HH // GW                   # 16 groups per sample
NSUB = GW // 128                # 4 subchunks per group

WIDTHS0 = [2048, 2048, 2048, 2048]
WIDTHS1 = [2048, 2048, 2048, 2048]
OUT_PIECES = [(0, 512), (512, 512), (1024, 1024), (2048, 2048),
              (4096, 2048), (6144, 2048)]

# groups whose text-half exp runs as DVE Schraudolph instead of Act exp
SPLIT_EXP = {0: set(range(0, 16)) - {0}, 1: {14, 15}}
# groups whose WHOLE exp (vis+text) is DVE Schraudolph
FULL_SCH = {0: set(), 1: set()}
# output pieces whose g*text mult runs on Act (big, tail pieces)
POOL_ADD = {0: {0, 1, 2, 4}, 1: set()}
# s0 phase2 is emitted after this s1 group's matmuls
PH2_0_AT = -1
# s0 output pieces emitted after these s1 groups
OT0_SCHED = {12: [0, 1], 13: [2, 3], 14: [4], 15: [5]}
# Schraudolph constants: bf16 bits of e^t ~= 184.6627*t + 16250.4
SCH_A = 184.6627
SCH_B = 16250.4


def _build_program():
    nc = bacc.Bacc()
    # [sample, channel, half, n]: sbuf partition p = h*64 + c
    vis_d = nc.declare_dram_parameter("vis", [2, 2, C, HH], F32,
                                      isOutput=False)
    text_d = nc.declare_dram_parameter("text", [2, 2, C, HH], F32,
                                       isOutput=False)
    out_d = nc.declare_dram_parameter("out", [2, 2, C, HH], BF16,
                                      isOutput=True)

    with ExitStack() as ctx:
        tc = ctx.enter_context(tile.TileContext(nc))
        _emit(ctx, tc, vis_d, text_d, out_d)
    nc.finalize()
    return nc


def _emit(ctx: ExitStack, tc: tile.TileContext, vis_d, text_d, out_d):
    nc = tc.nc

    big = ctx.enter_context(tc.tile_pool(name="big", bufs=2))
    est = ctx.enter_context(tc.tile_pool(name="est", bufs=12))
    xst = ctx.enter_context(tc.tile_pool(name="xst", bufs=12))
    ostage = ctx.enter_context(tc.tile_pool(name="ostage", bufs=5))
    consts = ctx.enter_context(tc.tile_pool(name="consts", bufs=1))
    small = ctx.enter_context(tc.tile_pool(name="small", bufs=2))
    tp = ctx.enter_context(tc.tile_pool(name="tp", bufs=5, space="PSUM"))
    jpool = ctx.enter_context(tc.tile_pool(name="jpool", bufs=2,
                                           space="PSUM"))
    p2psum = ctx.enter_context(tc.tile_pool(name="p2psum", bufs=1,
                                            space="PSUM"))

    ident_bf = consts.tile([P, P], BF16)

    def load_sample(s, widths, after_first=None):
        vis_ch, text_ch, offs = [], [], []
        o = 0
        for k, w in enumerate(widths):
            vt = big.tile([P, w], BF16, tag=f"vis{k}", name=f"vis{s}_{k}")
            tt = big.tile([P, w], BF16, tag=f"text{k}", name=f"text{s}_{k}")
            src_v = vis_d[s, :, :, o:o + w].rearrange("h c n -> (h c) n")
            src_t = text_d[s, :, :, o:o + w].rearrange("h c n -> (h c) n")
            nc.gpsimd.dma_start(out=vt, in_=src_v)
            nc.gpsimd.dma_start(out=tt, in_=src_t)
            if k == 0 and after_first is not None:
                after_first()
            vis_ch.append(vt)
            text_ch.append(tt)
            offs.append(o)
            o += w
        return vis_ch, text_ch, offs

    # identity first, then keep the PE continuously busy with rotated
    # dummy transposes so the first real transposes run at full p-state.
    make_identity(nc, ident_bf)
    warm = p2psum.tile([16, 256], BF16, tag="p2a", name="warm")
    for r in range(30):
        half = (r % 2) * 128
        nc.tensor.transpose(warm[:, half:half + 128],
                            ident_bf[0:P, 0:16], ident_bf)

    tiles = [load_sample(0, WIDTHS0),
             load_sample(1, WIDTHS1)]
    widths_all = [WIDTHS0, WIDTHS1]

    ones_bf = consts.tile([P, 1], BF16)
    nc.vector.memset(ones_bf, 1.0)
    ones64 = consts.tile([C, C], F32)
    nc.vector.memset(ones64, 1.0)
    eps_ap = consts.tile([C, 1], F32)
    nc.vector.memset(eps_ap, EPS)
    nkc_ap = consts.tile([C, 1], F32)
    nc.vector.memset(nkc_ap, -(1.0 + HW * EPS))

    def chunk_of(s, o, w):
        offs, widths = tiles[s][2], widths_all[s]
        k = next(i for i in range(len(offs))
                 if offs[i] <= o and o + w <= offs[i] + widths[i])
        return k, o - offs[k]

    # per-(sample, group) state
    rawt_t = {}
    xe_t = {}
    ee_t = {}
    j_t = {}
    sums_t = {}
    ph2 = {}

    def emit_sample_psum(s):
        js = jpool.tile([P, P + 8], F32, tag="jsums", name=f"jsums{s}")
        j_t[s] = js[:, 0:P]
        sums_t[s] = js[:, P:P + 4]

    def emit_trans(s, g):
        vis_ch, text_ch, _ = tiles[s]
        o = g * GW
        kv, lo = chunk_of(s, o, GW)
        rawt = tp.tile([P, 2 * GW], BF16, tag="rawt", name=f"rawt{s}_{g}")
        for i in range(NSUB):
            nc.tensor.transpose(rawt[:, i * 128:(i + 1) * 128],
                                vis_ch[kv][:, lo + i * 128:
                                           lo + (i + 1) * 128],
                                ident_bf)
            nc.tensor.transpose(rawt[:, GW + i * 128:GW + (i + 1) * 128],
                                text_ch[kv][:, lo + i * 128:
                                            lo + (i + 1) * 128],
                                ident_bf)
        rawt_t[(s, g)] = rawt

    def emit_exp(s, g):
        rawt = rawt_t[(s, g)]
        ee = est.tile([P, 2 * GW], BF16, tag="ee", name=f"ee{s}_{g}")
        if g in FULL_SCH[s]:
            nc.vector.tensor_scalar(out=ee.bitcast(I16), in0=rawt,
                                    scalar1=SCH_A, scalar2=SCH_B,
                                    op0=ALU.mult, op1=ALU.add)
        elif g in SPLIT_EXP[s]:
            # vis half exact on Act; text half one-op Schraudolph on DVE
            nc.scalar.activation(out=ee[:, 0:GW], in_=rawt[:, 0:GW],
                                 func=AF.Exp)
            nc.vector.tensor_scalar(out=ee[:, GW:2 * GW].bitcast(I16),
                                    in0=rawt[:, GW:2 * GW],
                                    scalar1=SCH_A, scalar2=SCH_B,
                                    op0=ALU.mult, op1=ALU.add)
        else:
            nc.scalar.activation(out=ee, in_=rawt, func=AF.Exp)
        ee_t[(s, g)] = ee

    def emit_xe(s, g):
        rawt = rawt_t[(s, g)]
        ee = ee_t[(s, g)]
        xeT = xst.tile([P, GW], BF16, tag="xe", name=f"xe{s}_{g}")
        nc.vector.tensor_tensor(out=xeT, in0=rawt[:, GW:2 * GW],
                                in1=ee[:, GW:2 * GW], op=ALU.mult)
        return xeT

    def emit_mms(s, g, xeT):
        ee = ee_t[(s, g)]
        evT = ee[:, 0:GW]
        etT = ee[:, GW:2 * GW]
        j_ps, sums_ps = j_t[s], sums_t[s]
        first, last = g == 0, g == NG - 1
        for i in range(NSUB):
            sl = slice(i * 128, (i + 1) * 128)
            fi = first and i == 0
            la = last and i == NSUB - 1
            nc.tensor.matmul(j_ps, lhsT=evT[:, sl], rhs=etT[:, sl],
                             start=fi, stop=la, skip_group_check=True)
            nc.tensor.matmul(sums_ps[:, 0:1], lhsT=evT[:, sl],
                             rhs=ones_bf, start=fi, stop=la,
                             skip_group_check=True)
            nc.tensor.matmul(sums_ps[:, 1:2], lhsT=etT[:, sl],
                             rhs=ones_bf, start=fi, stop=la,
                             skip_group_check=True)
            nc.tensor.matmul(sums_ps[:, 2:3], lhsT=xeT[:, sl],
                             rhs=ones_bf, start=fi, stop=la,
                             skip_group_check=True)

    def emit_group(s, g):
        emit_exp(s, g)
        xe_t[(s, g)] = emit_xe(s, g)
        if g + 4 < NG:
            emit_trans(s, g + 4)
        if g > 0:
            emit_mms(s, g - 1, xe_t[(s, g - 1)])
        if g == NG - 1:
            emit_mms(s, g, xe_t[(s, g)])

    def ph2_ops(s):
        """Phase-2 as a list of single-op closures, dependency ordered."""
        st = {}

        def op_shi():
            st["shi"] = small.tile([C, 3], F32, tag="shi", name=f"shi{s}")
            nc.vector.tensor_copy(out=st["shi"], in_=sums_t[s][64:128, 0:3])

        def op_fold():
            st["sums"] = small.tile([C, 3], F32, tag="sums_sb",
                                    name=f"sums_sb{s}")
            nc.vector.tensor_tensor(out=st["sums"], in0=sums_t[s][0:64, 0:3],
                                    in1=st["shi"], op=ALU.add)

        def op_recip():
            st["recips"] = small.tile([C, 2], F32, tag="recips",
                                      name=f"recips{s}")
            nc.vector.reciprocal(out=st["recips"], in_=st["sums"][:, 0:2])

        def op_rsthw2():
            st["rsthw2"] = small.tile([C, 1], F32, tag="rsthw2",
                                      name=f"rsthw2{s}")
            nc.vector.tensor_scalar_mul(out=st["rsthw2"],
                                        in0=st["recips"][:, 1:2],
                                        scalar1=float(HW) * float(HW))

        def op_logst():
            st["logst"] = small.tile([C, 1], F32, tag="logst",
                                     name=f"logst{s}")
            nc.scalar.activation(out=st["logst"], in_=st["sums"][:, 1:2],
                                 func=AF.Ln)

        def op_negent():
            st["negent"] = small.tile([C, 1], F32, tag="negent",
                                      name=f"negent{s}")
            nc.vector.scalar_tensor_tensor(
                out=st["negent"], in0=st["sums"][:, 2:3],
                scalar=st["recips"][:, 1:2], in1=st["logst"],
                op0=ALU.mult, op1=ALU.subtract)

        def op_jhi():
            st["jhi"] = small.tile([C, C], BF16, tag="jhi", name=f"jhi{s}")
            nc.vector.tensor_copy(out=st["jhi"], in_=j_t[s][64:128, 64:128])

        def op_jfold():
            st["j64"] = small.tile([C, C], BF16, tag="j64", name=f"j64_{s}")
            nc.vector.tensor_tensor(out=st["j64"], in0=j_t[s][0:64, 0:64],
                                    in1=st["jhi"], op=ALU.add)

        def op_jscale():
            nc.vector.tensor_scalar_mul(out=st["j64"], in0=st["j64"],
                                        scalar1=st["recips"][:, 0:1])

        def op_jt():
            st["jt_ps"] = p2psum.tile([C, C], BF16, tag="p2a",
                                      name=f"jt{s}")
            nc.tensor.transpose(st["jt_ps"], st["j64"],
                                ident_bf[0:64, 0:64])

        def op_lterm():
            st["lterm"] = small.tile([C, C], F32, tag="lterm",
                                     name=f"lterm{s}")
            nc.scalar.activation(out=st["lterm"], in_=st["jt_ps"],
                                 func=AF.Ln, scale=st["rsthw2"],
                                 bias=eps_ap)

        def op_vm():
            st["vm"] = small.tile([C, C], F32, tag="vm", name=f"vm{s}")
            nc.vector.tensor_tensor(out=st["vm"], in0=st["jt_ps"],
                                    in1=st["lterm"], op=ALU.mult)

        def op_u():
            st["u"] = small.tile([C, 1], F32, tag="u", name=f"u{s}")
            nc.vector.tensor_reduce(out=st["u"], in_=st["vm"], axis=AX.X,
                                    op=ALU.add)

        def op_uscale():
            nc.vector.tensor_scalar(out=st["u"], in0=st["u"],
                                    scalar1=st["recips"][:, 1:2],
                                    scalar2=None, op0=ALU.mult)

        def op_mib():
            st["mib"] = sums_t[s].tensor[0:64, 132:133]
            nc.tensor.matmul(st["mib"], lhsT=ones64, rhs=st["u"],
                             start=True, stop=True, skip_group_check=True)

        def op_arg():
            st["arg"] = small.tile([C, 1], F32, tag="arg", name=f"arg{s}")
            nc.vector.scalar_tensor_tensor(
                out=st["arg"], in0=st["mib"], scalar=0.5, in1=st["negent"],
                op0=ALU.mult, op1=ALU.add)

        def op_gexp():
            st["g64"] = small.tile([C, 1], F32, tag="g64", name=f"g64{s}")
            nc.scalar.activation(out=st["g64"], in_=st["arg"], func=AF.Exp,
                                 scale=-1.0, bias=nkc_ap)

        def op_gadd():
            nc.vector.tensor_scalar_add(out=st["g64"], in0=st["g64"],
                                        scalar1=1.0)

        def op_grecip():
            nc.vector.reciprocal(out=st["g64"], in_=st["g64"])

        def op_gv():
            gv = small.tile([P, 1], F32, tag="g", name=f"g{s}")
            nc.vector.tensor_copy(out=gv[0:64, :], in_=st["g64"])
            nc.vector.tensor_copy(out=gv[64:128, :], in_=st["g64"])
            ph2[s] = gv

        return [op_shi, op_fold, op_recip, op_rsthw2, op_logst, op_negent,
                op_jhi, op_jfold, op_jscale, op_jt, op_lterm, op_vm,
                op_u, op_uscale, op_mib, op_arg, op_gexp, op_gadd,
                op_grecip, op_gv]

    def emit_phase2(s):
        for op in ph2_ops(s):
            op()

    def emit_ot(s, pi):
        vis_ch, text_ch, _ = tiles[s]
        gv = ph2[s]
        o, w = OUT_PIECES[pi]
        k, lo = chunk_of(s, o, w)
        gt = ostage.tile([P, w], BF16, tag="gt", name=f"gt{s}_{pi}")
        nc.vector.tensor_scalar(out=gt, in0=text_ch[k][:, lo:lo + w],
                                scalar1=gv, scalar2=None, op0=ALU.mult)
        ot = ostage.tile([P, w], BF16, tag="o", name=f"o{s}_{pi}")
        add_eng = nc.gpsimd if pi in POOL_ADD[s] else nc.vector
        add_eng.tensor_tensor(out=ot, in0=gt,
                              in1=vis_ch[k][:, lo:lo + w], op=ALU.add)
        dst = out_d[s, :, :, o:o + w].rearrange("h c n -> (h c) n")
        nc.sync.dma_start(out=dst, in_=ot)

    # ---- interleaved emission ----    # ---- interleaved emission ----
    emit_sample_psum(0)
    for gg in range(4):
        emit_trans(0, gg)
    for g in range(NG):
        emit_group(0, g)

    emit_sample_psum(1)
    for gg in range(4):
        emit_trans(1, gg)
    s0_tail = list(ph2_ops(0))
    s0_ot_done = [False]

    def drain(k):
        while k > 0 and s0_tail:
            s0_tail.pop(0)()
            k -= 1

    for g in range(NG):
        emit_exp(1, g)
        drain(2)
        xe_t[(1, g)] = emit_xe(1, g)
        if g + 4 < NG:
            emit_trans(1, g + 4)
        if g > 0:
            emit_mms(1, g - 1, xe_t[(1, g - 1)])
        if g == NG - 1:
            emit_mms(1, g, xe_t[(1, g)])
        drain(2)
        if g == 11:
            drain(len(s0_tail))

    drain(len(s0_tail))
    # weave s0's output pass into ph2(1)'s dependency-chain gaps
    ot0 = list(range(len(OUT_PIECES)))
    for i, op in enumerate(ph2_ops(1)):
        op()
        if ot0:
            emit_ot(0, ot0.pop(0))
    while ot0:
        emit_ot(0, ot0.pop(0))
    for pi in [2, 3, 4, 5, 1, 0]:
        emit_ot(1, pi)


_PROGRAM = None


def _get_program():
    global _PROGRAM
    if _PROGRAM is None:
        _PROGRAM = _build_program()
    return _PROGRAM


def kernel(vis_feat: np.ndarray, text_feat: np.ndarray) -> np.ndarray:
    nc = _get_program()
    vis = np.ascontiguousarray(vis_feat, dtype=np.float32)
    text = np.ascontiguousarray(text_feat, dtype=np.float32)
    bpc = B // NCORES
    in_maps = [
        {
            "vis": np.ascontiguousarray(
                vis[i * bpc:(i + 1) * bpc].reshape(bpc, C, 2, HH)
                .transpose(0, 2, 1, 3)),
            "text": np.ascontiguousarray(
                text[i * bpc:(i + 1) * bpc].reshape(bpc, C, 2, HH)
                .transpose(0, 2, 1, 3)),
        }
        for i in range(NCORES)
    ]
    res = run_bass_kernel_spmd(nc, in_maps, list(range(NCORES)))
    out = np.concatenate(
        [np.asarray(r["out"]).astype(np.float32)
         .reshape(bpc, 2, C, HH).transpose(0, 2, 1, 3).reshape(bpc, C, H, W)
         for r in res.results],
        axis=0)
    return out


# revision 65
# speedup vs baseline: 1.0139x; 1.0139x over previous
"""EntropyGuidance Trainium2 kernel — bf16 I/O, transpose-then-exp, staggered.

Layout: per sample [128 partitions = (h c): p = h*64 + c, 8192 free] bf16.
DRAM output is bf16 (host upcasts), loads cast f32->bf16 in the DGE, so
every DMA is billed on its 2-byte side. fp32 is kept for statistics.

Per 512-column group g of each sample:
  1. PE transposes raw vis|text subchunks into one PSUM tile [128, 1024].
  2. exp reads the PSUM tile and writes transposed exponentials straight
     to SBUF: on the Activation engine normally, or as a one-op DVE
     Schraudolph (i16 = 184.6627*t + 16250.4, bitcast to bf16; ~3% rel
     err, only ever used by the guide-weight path which tolerates it)
     for groups listed in DVE_EXP — this drains the Act queue faster.
  3. J += evT^T @ etT per 128-column slice (PSUM fp32).
  4. Sv/St/T ride near-free PE matmuls with a [128,1] ones rhs
     (T uses xeT = rawT_text * etT, one DVE mult per group).

Folds (h c) -> c use a base-partition-changing copy plus a PSUM+SBUF add
(both verified legal); the guide is computed on 64 partitions and
broadcast with two copies. Samples are staggered: s0's phase-2 and
output pass are emitted interleaved into s1's group stream so no engine
queue head-of-line blocks the other sample.
"""

import sys

sys.path.insert(0, "/opt/trn_rl_repo")

from contextlib import ExitStack

import numpy as np

import concourse.bacc as bacc
import concourse.tile as tile
from concourse import mybir
from concourse.bass_utils import run_bass_kernel_spmd
from concourse.masks import make_identity

_orig_get_act_tables = bacc.get_activation_tables


def _lnexp_only_tables(module_arch):
    tabs = _orig_get_act_tables(module_arch)
    return {
        name: (funcs if name == "natural_log_exp_and_others" else set())
        for name, funcs in tabs.items()
    }


bacc.get_activation_tables = _lnexp_only_tables

F32 = mybir.dt.float32
BF16 = mybir.dt.bfloat16
I16 = mybir.dt.int16
AF = mybir.ActivationFunctionType
ALU = mybir.AluOpType
AX = mybir.AxisListType

B, C, H, W = 16, 64, 128, 128
HW = H * W                      # 16384
HH = HW // 2                    # 8192 per half
NCORES = 8
P = 128                         # partitions = (h c)
EPS = 1e-9
GW = 512                        # n-columns per group
NG = HH // GW                   # 16 groups per sample
NSUB = GW // 128                # 4 subchunks per group

WIDTHS0 = [2048, 2048, 2048, 2048]
WIDTHS1 = [2048, 2048, 2048, 2048]
OUT_PIECES = [(0, 512), (512, 512), (1024, 1024), (2048, 2048),
              (4096, 2048), (6144, 2048)]

# groups whose text-half exp runs as DVE Schraudolph instead of Act exp
SPLIT_EXP = {0: set(range(0, 16)) - {0}, 1: set()}
# groups whose WHOLE exp (vis+text) is DVE Schraudolph
FULL_SCH = {0: set(), 1: {14, 15}}
# output pieces whose g*text mult runs on Act (big, tail pieces)
POOL_ADD = {0: {0, 1, 2, 4}, 1: set()}
# s0 phase2 is emitted after this s1 group's matmuls
PH2_0_AT = -1
# s0 output pieces emitted after these s1 groups
OT0_SCHED = {12: [0, 1], 13: [2, 3], 14: [4], 15: [5]}
# Schraudolph constants: bf16 bits of e^t ~= 184.6627*t + 16250.4
SCH_A = 184.6627
SCH_B = 16250.4


def _build_program():
    nc = bacc.Bacc()
    # [sample, channel, half, n]: sbuf partition p = h*64 + c
    vis_d = nc.declare_dram_parameter("vis", [2, 2, C, HH], F32,
                                      isOutput=False)
    text_d = nc.declare_dram_parameter("text", [2, 2, C, HH], F32,
                                       isOutput=False)
    out_d = nc.declare_dram_parameter("out", [2, 2, C, HH], BF16,
                                      isOutput=True)

    with ExitStack() as ctx:
        tc = ctx.enter_context(tile.TileContext(nc))
        _emit(ctx, tc, vis_d, text_d, out_d)
    nc.finalize()
    return nc


def _emit(ctx: ExitStack, tc: tile.TileContext, vis_d, text_d, out_d):
    nc = tc.nc

    big = ctx.enter_context(tc.tile_pool(name="big", bufs=2))
    est = ctx.enter_context(tc.tile_pool(name="est", bufs=12))
    xst = ctx.enter_context(tc.tile_pool(name="xst", bufs=12))
    ostage = ctx.enter_context(tc.tile_pool(name="ostage", bufs=8))
    consts = ctx.enter_context(tc.tile_pool(name="consts", bufs=1))
    small = ctx.enter_context(tc.tile_pool(name="small", bufs=2))
    tp = ctx.enter_context(tc.tile_pool(name="tp", bufs=5, space="PSUM"))
    jpool = ctx.enter_context(tc.tile_pool(name="jpool", bufs=2,
                                           space="PSUM"))
    p2psum = ctx.enter_context(tc.tile_pool(name="p2psum", bufs=1,
                                            space="PSUM"))

    ident_bf = consts.tile([P, P], BF16)

    def load_sample(s, widths, after_first=None):
        vis_ch, text_ch, offs = [], [], []
        o = 0
        for k, w in enumerate(widths):
            vt = big.tile([P, w], BF16, tag=f"vis{k}", name=f"vis{s}_{k}")
            tt = big.tile([P, w], BF16, tag=f"text{k}", name=f"text{s}_{k}")
            src_v = vis_d[s, :, :, o:o + w].rearrange("h c n -> (h c) n")
            src_t = text_d[s, :, :, o:o + w].rearrange("h c n -> (h c) n")
            nc.gpsimd.dma_start(out=vt, in_=src_v)
            nc.gpsimd.dma_start(out=tt, in_=src_t)
            if k == 0 and after_first is not None:
                after_first()
            vis_ch.append(vt)
            text_ch.append(tt)
            offs.append(o)
            o += w
        return vis_ch, text_ch, offs

    # identity first, then keep the PE continuously busy with rotated
    # dummy transposes so the first real transposes run at full p-state.
    make_identity(nc, ident_bf)
    warm = p2psum.tile([16, 256], BF16, tag="p2a", name="warm")
    for r in range(30):
        half = (r % 2) * 128
        nc.tensor.transpose(warm[:, half:half + 128],
                            ident_bf[0:P, 0:16], ident_bf)

    tiles = [load_sample(0, WIDTHS0),
             load_sample(1, WIDTHS1)]
    widths_all = [WIDTHS0, WIDTHS1]

    ones_bf = consts.tile([P, 1], BF16)
    nc.vector.memset(ones_bf, 1.0)
    ones64 = consts.tile([C, C], F32)
    nc.vector.memset(ones64, 1.0)
    eps_ap = consts.tile([C, 1], F32)
    nc.vector.memset(eps_ap, EPS)
    nkc_ap = consts.tile([C, 1], F32)
    nc.vector.memset(nkc_ap, -(1.0 + HW * EPS))

    def chunk_of(s, o, w):
        offs, widths = tiles[s][2], widths_all[s]
        k = next(i for i in range(len(offs))
                 if offs[i] <= o and o + w <= offs[i] + widths[i])
        return k, o - offs[k]

    # per-(sample, group) state
    rawt_t = {}
    xe_t = {}
    ee_t = {}
    j_t = {}
    sums_t = {}
    ph2 = {}

    def emit_sample_psum(s):
        js = jpool.tile([P, P + 8], F32, tag="jsums", name=f"jsums{s}")
        j_t[s] = js[:, 0:P]
        sums_t[s] = js[:, P:P + 4]

    def emit_trans(s, g):
        vis_ch, text_ch, _ = tiles[s]
        o = g * GW
        kv, lo = chunk_of(s, o, GW)
        rawt = tp.tile([P, 2 * GW], BF16, tag="rawt", name=f"rawt{s}_{g}")
        for i in range(NSUB):
            nc.tensor.transpose(rawt[:, i * 128:(i + 1) * 128],
                                vis_ch[kv][:, lo + i * 128:
                                           lo + (i + 1) * 128],
                                ident_bf)
            nc.tensor.transpose(rawt[:, GW + i * 128:GW + (i + 1) * 128],
                                text_ch[kv][:, lo + i * 128:
                                            lo + (i + 1) * 128],
                                ident_bf)
        rawt_t[(s, g)] = rawt

    def emit_exp(s, g):
        rawt = rawt_t[(s, g)]
        ee = est.tile([P, 2 * GW], BF16, tag="ee", name=f"ee{s}_{g}")
        if g in FULL_SCH[s]:
            nc.vector.tensor_scalar(out=ee.bitcast(I16), in0=rawt,
                                    scalar1=SCH_A, scalar2=SCH_B,
                                    op0=ALU.mult, op1=ALU.add)
        elif g in SPLIT_EXP[s]:
            # vis half exact on Act; text half one-op Schraudolph on DVE
            nc.scalar.activation(out=ee[:, 0:GW], in_=rawt[:, 0:GW],
                                 func=AF.Exp)
            nc.vector.tensor_scalar(out=ee[:, GW:2 * GW].bitcast(I16),
                                    in0=rawt[:, GW:2 * GW],
                                    scalar1=SCH_A, scalar2=SCH_B,
                                    op0=ALU.mult, op1=ALU.add)
        else:
            nc.scalar.activation(out=ee, in_=rawt, func=AF.Exp)
        ee_t[(s, g)] = ee

    def emit_xe(s, g):
        rawt = rawt_t[(s, g)]
        ee = ee_t[(s, g)]
        xeT = xst.tile([P, GW], BF16, tag="xe", name=f"xe{s}_{g}")
        nc.vector.tensor_tensor(out=xeT, in0=rawt[:, GW:2 * GW],
                                in1=ee[:, GW:2 * GW], op=ALU.mult)
        return xeT

    def emit_mms(s, g, xeT):
        ee = ee_t[(s, g)]
        evT = ee[:, 0:GW]
        etT = ee[:, GW:2 * GW]
        j_ps, sums_ps = j_t[s], sums_t[s]
        first, last = g == 0, g == NG - 1
        for i in range(NSUB):
            sl = slice(i * 128, (i + 1) * 128)
            fi = first and i == 0
            la = last and i == NSUB - 1
            nc.tensor.matmul(j_ps, lhsT=evT[:, sl], rhs=etT[:, sl],
                             start=fi, stop=la, skip_group_check=True)
            nc.tensor.matmul(sums_ps[:, 0:1], lhsT=evT[:, sl],
                             rhs=ones_bf, start=fi, stop=la,
                             skip_group_check=True)
            nc.tensor.matmul(sums_ps[:, 1:2], lhsT=etT[:, sl],
                             rhs=ones_bf, start=fi, stop=la,
                             skip_group_check=True)
            nc.tensor.matmul(sums_ps[:, 2:3], lhsT=xeT[:, sl],
                             rhs=ones_bf, start=fi, stop=la,
                             skip_group_check=True)

    def emit_group(s, g):
        emit_exp(s, g)
        xe_t[(s, g)] = emit_xe(s, g)
        if g + 4 < NG:
            emit_trans(s, g + 4)
        if g > 0:
            emit_mms(s, g - 1, xe_t[(s, g - 1)])
        if g == NG - 1:
            emit_mms(s, g, xe_t[(s, g)])

    def ph2_ops(s):
        """Phase-2 as a list of single-op closures, dependency ordered."""
        st = {}

        def op_shi():
            st["shi"] = small.tile([C, 3], F32, tag="shi", name=f"shi{s}")
            nc.vector.tensor_copy(out=st["shi"], in_=sums_t[s][64:128, 0:3])

        def op_fold():
            st["sums"] = small.tile([C, 3], F32, tag="sums_sb",
                                    name=f"sums_sb{s}")
            nc.vector.tensor_tensor(out=st["sums"], in0=sums_t[s][0:64, 0:3],
                                    in1=st["shi"], op=ALU.add)

        def op_recip():
            st["recips"] = small.tile([C, 2], F32, tag="recips",
                                      name=f"recips{s}")
            nc.vector.reciprocal(out=st["recips"], in_=st["sums"][:, 0:2])

        def op_rsthw2():
            st["rsthw2"] = small.tile([C, 1], F32, tag="rsthw2",
                                      name=f"rsthw2{s}")
            nc.vector.tensor_scalar_mul(out=st["rsthw2"],
                                        in0=st["recips"][:, 1:2],
                                        scalar1=float(HW) * float(HW))

        def op_logst():
            st["logst"] = small.tile([C, 1], F32, tag="logst",
                                     name=f"logst{s}")
            nc.scalar.activation(out=st["logst"], in_=st["sums"][:, 1:2],
                                 func=AF.Ln)

        def op_negent():
            st["negent"] = small.tile([C, 1], F32, tag="negent",
                                      name=f"negent{s}")
            nc.vector.scalar_tensor_tensor(
                out=st["negent"], in0=st["sums"][:, 2:3],
                scalar=st["recips"][:, 1:2], in1=st["logst"],
                op0=ALU.mult, op1=ALU.subtract)

        def op_jhi():
            st["jhi"] = small.tile([C, C], BF16, tag="jhi", name=f"jhi{s}")
            nc.vector.tensor_copy(out=st["jhi"], in_=j_t[s][64:128, 64:128])

        def op_jfold():
            st["j64"] = small.tile([C, C], BF16, tag="j64", name=f"j64_{s}")
            nc.vector.tensor_tensor(out=st["j64"], in0=j_t[s][0:64, 0:64],
                                    in1=st["jhi"], op=ALU.add)

        def op_jscale():
            nc.vector.tensor_scalar_mul(out=st["j64"], in0=st["j64"],
                                        scalar1=st["recips"][:, 0:1])

        def op_jt():
            st["jt_ps"] = p2psum.tile([C, C], BF16, tag="p2a",
                                      name=f"jt{s}")
            nc.tensor.transpose(st["jt_ps"], st["j64"],
                                ident_bf[0:64, 0:64])

        def op_lterm():
            st["lterm"] = small.tile([C, C], F32, tag="lterm",
                                     name=f"lterm{s}")
            nc.scalar.activation(out=st["lterm"], in_=st["jt_ps"],
                                 func=AF.Ln, scale=st["rsthw2"],
                                 bias=eps_ap)

        def op_vm():
            st["vm"] = small.tile([C, C], F32, tag="vm", name=f"vm{s}")
            nc.vector.tensor_tensor(out=st["vm"], in0=st["jt_ps"],
                                    in1=st["lterm"], op=ALU.mult)

        def op_u():
            st["u"] = small.tile([C, 1], F32, tag="u", name=f"u{s}")
            nc.vector.tensor_reduce(out=st["u"], in_=st["vm"], axis=AX.X,
                                    op=ALU.add)

        def op_uscale():
            nc.vector.tensor_scalar(out=st["u"], in0=st["u"],
                                    scalar1=st["recips"][:, 1:2],
                                    scalar2=None, op0=ALU.mult)

        def op_mib():
            st["mib"] = sums_t[s].tensor[0:64, 132:133]
            nc.tensor.matmul(st["mib"], lhsT=ones64, rhs=st["u"],
                             start=True, stop=True, skip_group_check=True)

        def op_arg():
            st["arg"] = small.tile([C, 1], F32, tag="arg", name=f"arg{s}")
            nc.vector.scalar_tensor_tensor(
                out=st["arg"], in0=st["mib"], scalar=0.5, in1=st["negent"],
                op0=ALU.mult, op1=ALU.add)

        def op_gexp():
            st["g64"] = small.tile([C, 1], F32, tag="g64", name=f"g64{s}")
            nc.scalar.activation(out=st["g64"], in_=st["arg"], func=AF.Exp,
                                 scale=-1.0, bias=nkc_ap)

        def op_gadd():
            nc.vector.tensor_scalar_add(out=st["g64"], in0=st["g64"],
                                        scalar1=1.0)

        def op_grecip():
            nc.vector.reciprocal(out=st["g64"], in_=st["g64"])

        def op_gv():
            gv = small.tile([P, 1], F32, tag="g", name=f"g{s}")
            nc.vector.tensor_copy(out=gv[0:64, :], in_=st["g64"])
            nc.vector.tensor_copy(out=gv[64:128, :], in_=st["g64"])
            ph2[s] = gv

        return [op_shi, op_fold, op_recip, op_rsthw2, op_logst, op_negent,
                op_jhi, op_jfold, op_jscale, op_jt, op_lterm, op_vm,
                op_u, op_uscale, op_mib, op_arg, op_gexp, op_gadd,
                op_grecip, op_gv]

    def emit_phase2(s):
        for op in ph2_ops(s):
            op()

    def emit_ot(s, pi):
        vis_ch, text_ch, _ = tiles[s]
        gv = ph2[s]
        o, w = OUT_PIECES[pi]
        k, lo = chunk_of(s, o, w)
        gt = ostage.tile([P, w], BF16, tag="gt", name=f"gt{s}_{pi}")
        nc.vector.tensor_scalar(out=gt, in0=text_ch[k][:, lo:lo + w],
                                scalar1=gv, scalar2=None, op0=ALU.mult)
        ot = ostage.tile([P, w], BF16, tag="o", name=f"o{s}_{pi}")
        add_eng = nc.gpsimd if pi in POOL_ADD[s] else nc.vector
        add_eng.tensor_tensor(out=ot, in0=gt,
                              in1=vis_ch[k][:, lo:lo + w], op=ALU.add)
        dst = out_d[s, :, :, o:o + w].rearrange("h c n -> (h c) n")
        nc.sync.dma_start(out=dst, in_=ot)

    # ---- interleaved emission ----    # ---- interleaved emission ----
    emit_sample_psum(0)
    for gg in range(4):
        emit_trans(0, gg)
    for g in range(NG):
        emit_group(0, g)

    emit_sample_psum(1)
    for gg in range(4):
        emit_trans(1, gg)
    s0_tail = list(ph2_ops(0))
    s0_ot_done = [False]

    def drain(k):
        while k > 0 and s0_tail:
            s0_tail.pop(0)()
            k -= 1

    for g in range(NG):
        emit_exp(1, g)
        drain(2)
        xe_t[(1, g)] = emit_xe(1, g)
        if g + 4 < NG:
            emit_trans(1, g + 4)
        if g > 0:
            emit_mms(1, g - 1, xe_t[(1, g - 1)])
        if g == NG - 1:
            emit_mms(1, g, xe_t[(1, g)])
        drain(2)
        if g == 11:
            drain(len(s0_tail))

    drain(len(s0_tail))
    # weave s0's output pass into ph2(1)'s dependency-chain gaps
    ot0 = list(range(len(OUT_PIECES)))
    for i, op in enumerate(ph2_ops(1)):
        op()
        if ot0:
            emit_ot(0, ot0.pop(0))
    while ot0:
        emit_ot(0, ot0.pop(0))
    for pi in [2, 3, 4, 5, 1, 0]:
        emit_ot(1, pi)


_PROGRAM = None


def _get_program():
    global _PROGRAM
    if _PROGRAM is None:
        _PROGRAM = _build_program()
    return _PROGRAM


def kernel(vis_feat: np.ndarray, text_feat: np.ndarray) -> np.ndarray:
    nc = _get_program()
    vis = np.ascontiguousarray(vis_feat, dtype=np.float32)
    text = np.ascontiguousarray(text_feat, dtype=np.float32)
    bpc = B // NCORES
    in_maps = [
        {
            "vis": np.ascontiguousarray(
                vis[i * bpc:(i + 1) * bpc].reshape(bpc, C, 2, HH)
                .transpose(0, 2, 1, 3)),
            "text": np.ascontiguousarray(
                text[i * bpc:(i + 1) * bpc].reshape(bpc, C, 2, HH)
                .transpose(0, 2, 1, 3)),
        }
        for i in range(NCORES)
    ]
    res = run_bass_kernel_spmd(nc, in_maps, list(range(NCORES)))
    out = np.concatenate(
        [np.asarray(r["out"]).astype(np.float32)
         .reshape(bpc, 2, C, HH).transpose(0, 2, 1, 3).reshape(bpc, C, H, W)
         for r in res.results],
        axis=0)
    return out
